# revision 1
# baseline (speedup 1.0000x reference)
"""Char-LSTM kernel for Trainium2 (8 NeuronCores, data parallel).

Strategy
--------
Host side:
  * Precompute G = emb @ W_ih.T + b_ih + b_hh  (vocab=100 -> [100, 4H]).
    The per-step embedding+input-projection then becomes a gather of G rows,
    which we realize on-device as an exact one-hot matmul accumulating
    directly into the same PSUM region as the recurrent matmul.
  * Sort words by length, deal them into per-core blocks of 512 words of a
    single length each (padded with dummies); leftovers go to "overflow"
    blocks which run all 16 steps with per-step h capture.
  * Blocks are paired into groups of 1024 words: block A lives on SBUF
    partitions 0:64, block B on 64:128 (state stored transposed, [H, words]).

Device side (identical SPMD program on all 8 cores):
  Per group-step:
    * 8 one-hot matmuls (vocab split 0:64 / 64:100 across PE row-groups) and
      8 recurrent matmuls (K=64), M=64 each, accumulating into one
      [128, 2048] PSUM tile laid out as banks [i | f | o | g] with block A in
      partitions 0:64 and block B in 64:128.
    * One sigmoid over [128, 1536] (i,f,o), tanh over g, then the cell update
      on the Vector engine, tanh(c) and h = o * tanh(c).
  Groups are emitted interleaved ~3 wide so the recurrence chains of
  independent groups pipeline across the Tensor/Scalar/Vector engines.
"""

import os
import sys

for _p in ("/opt/trn_rl_repo", "/root/.axon_site/_ro/trn_rl_repo"):
    if os.path.isdir(_p) and _p not in sys.path:
        sys.path.insert(0, _p)

import numpy as np
import ml_dtypes

BF16 = ml_dtypes.bfloat16

H = 64          # hidden size
E = 32          # char embedding size
V = 100         # vocab
MAXL = 16       # max word length
BLK = 512       # words per block (one half of a group)
NCORES = 8
GATE4 = 4 * H   # 256

# torch gate order in the weights is [i, f, g, o]; we stage banks as
# [i, f, o, g] so sigmoid covers one contiguous [128, 1536] span.
_GATE_PERM = np.concatenate([
    np.arange(0, 64),        # i
    np.arange(64, 128),      # f
    np.arange(192, 256),     # o
    np.arange(128, 192),     # g
])

INTERLEAVE = int(__import__("os").environ.get("LSTM_INTERLEAVE", "3"))
SKIPB = __import__("os").environ.get("LSTM_SKIPB", "1") == "1"
_PROGRAM_CACHE = {}


# --------------------------------------------------------------------------
# Host-side planning
# --------------------------------------------------------------------------

def _plan(lengths):
    """Assign words to (core, block, column) slots.

    Returns dict with:
      blocks: list (shared across cores) of dicts {L, is_ov, ov_idx}
      groups: list of dicts {a, b, steps} (block indices)
      sched:  emission order list of (group_idx, t)
      assign: per core: list of np arrays [BLK] of word ids (-1 = dummy),
              aligned with blocks
    """
    n = lengths.shape[0]
    lengths = lengths.astype(np.int64)
    order = np.argsort(lengths, kind="stable")

    per_core_words = [[] for _ in range(NCORES)]   # per core: list of [BLK] arrays
    block_meta = []                                # shared: (L, is_ov)

    leftovers = []
    for L in range(1, MAXL + 1):
        idx = order[np.searchsorted(lengths, L, side="left", sorter=order):
                    np.searchsorted(lengths, L, side="right", sorter=order)]
        take = idx[: NCORES * BLK]
        leftovers.append(idx[NCORES * BLK:])
        arr = np.full(NCORES * BLK, -1, dtype=np.int64)
        arr[: take.shape[0]] = take
        arr = arr.reshape(NCORES, BLK)
        for c in range(NCORES):
            per_core_words[c].append(arr[c])
        block_meta.append((L, False))

    leftovers = np.concatenate(leftovers) if leftovers else np.empty(0, np.int64)

    # Try to fold leftover words into the free slots of the length-16 block
    # (which then runs per-step capture); fall back to dedicated overflow
    # blocks when they don't fit.
    l16 = MAXL - 1  # index of the length-16 block in block_meta order
    free16 = [int((per_core_words[c][l16] < 0).sum()) for c in range(NCORES)]
    if leftovers.shape[0] <= sum(free16):
        block_meta[l16] = (MAXL, True)
        pos = 0
        for c in range(NCORES):
            k = min(free16[c], leftovers.shape[0] - pos)
            if k > 0:
                arr = per_core_words[c][l16]
                slots = np.nonzero(arr < 0)[0][:k]
                arr[slots] = leftovers[pos:pos + k]
                pos += k
        leftovers = leftovers[:0]

    if leftovers.shape[0]:
        n_ov = -(-leftovers.shape[0] // (NCORES * BLK))
        ov = np.full(n_ov * NCORES * BLK, -1, dtype=np.int64)
        ov[: leftovers.shape[0]] = leftovers
        ov = ov.reshape(n_ov, NCORES, BLK)
        for i in range(n_ov):
            for c in range(NCORES):
                per_core_words[c].append(ov[i, c])
            block_meta.append((MAXL, True))

    if len(block_meta) % 2 == 1:
        for c in range(NCORES):
            per_core_words[c].append(np.full(BLK, -1, dtype=np.int64))
        block_meta.append((1, False))

    # Sort blocks: descending length, overflow blocks first among equals so
    # they pair with the longest regular block.
    nb = len(block_meta)
    key = sorted(range(nb), key=lambda b: (-block_meta[b][0], not block_meta[b][1]))
    blocks = []
    ov_count = 0
    for b in key:
        L, is_ov = block_meta[b]
        blocks.append({"L": L, "is_ov": is_ov,
                       "ov_idx": (ov_count if is_ov else -1), "orig": b})
        if is_ov:
            ov_count += 1

    assign = [[per_core_words[c][blocks[i]["orig"]] for i in range(nb)]
              for c in range(NCORES)]

    groups = []
    for i in range(0, nb, 2):
        groups.append({"a": i, "b": i + 1,
                       "steps": max(blocks[i]["L"], blocks[i + 1]["L"])})

    # Greedy interleaved schedule: each round, one step of the (up to) 3
    # groups with the most remaining work.
    remaining = [g["steps"] for g in groups]
    next_t = [0] * len(groups)
    sched = []
    while any(r > 0 for r in remaining):
        act = sorted(range(len(groups)), key=lambda g: -remaining[g])[:INTERLEAVE]
        act = [g for g in act if remaining[g] > 0]
        for g in act:
            sched.append((g, next_t[g]))
            next_t[g] += 1
            remaining[g] -= 1

    # capture steps: for each capture block, the union (over cores) of
    # final steps of its words with length < MAXL, plus MAXL-1 (so length-16
    # words folded into a capture block are also covered).
    for bi, blk in enumerate(blocks):
        if not blk["is_ov"]:
            continue
        steps = set()
        for c in range(NCORES):
            w = assign[c][bi]
            w = w[w >= 0]
            steps.update((lengths[w] - 1).tolist())
        blk["cap_steps"] = tuple(sorted(steps))

    return {"blocks": blocks, "groups": groups, "sched": sched,
            "assign": assign, "n_ov": ov_count}


def _build_onehots(plan, chars, lengths):
    """Per-core one-hot slab tensors [n_slabs, V, BLK] float32.

    Slab order matches the device program's emission order: for each
    scheduled (group, t): A half then B half.
    """
    blocks, groups, sched = plan["blocks"], plan["groups"], plan["sched"]
    n_slabs = 2 * len(sched)
    out = []
    for c in range(NCORES):
        oh = np.zeros((n_slabs, V, BLK), dtype=BF16)
        slab = 0
        for (g, t) in sched:
            for blk_idx in (groups[g]["a"], groups[g]["b"]):
                words = plan["assign"][c][blk_idx]
                valid = (words >= 0)
                w = words[valid]
                if w.shape[0]:
                    alive = t < lengths[w]
                    cols = np.nonzero(valid)[0][alive]
                    ch = chars[w[alive], t]
                    oh[slab, ch, cols] = 1.0
                slab += 1
        out.append(oh)
    return out


# --------------------------------------------------------------------------
# Device program
# --------------------------------------------------------------------------

def _build_program(plan_sig, blocks, groups, sched, n_ov, variant="full",
                   reps=1):
    import concourse.bass as bass
    import concourse.tile as tile
    from concourse import bacc, mybir
    from contextlib import nullcontext

    do_mm = variant not in ("nomm", "onemm")
    one_mm = variant == "onemm"
    do_act = variant not in ("noact",)
    do_dma = variant not in ("nodma",)

    f32 = mybir.dt.float32
    bf16 = mybir.dt.bfloat16
    n_blocks = len(blocks)
    n_slabs = 2 * len(sched)

    nc = bacc.Bacc("TRN2", target_bir_lowering=False, debug=False,
                   num_devices=NCORES)
    oh_d = nc.dram_tensor("oh", [n_slabs, V, BLK], bf16, kind="ExternalInput")
    gtab_d = nc.dram_tensor("gtab", [128, GATE4], bf16, kind="ExternalInput")
    whha_d = nc.dram_tensor("whha", [128, GATE4], bf16, kind="ExternalInput")
    whhb_d = nc.dram_tensor("whhb", [128, GATE4], bf16, kind="ExternalInput")
    out_d = nc.dram_tensor("out", [n_blocks, H, BLK], f32, kind="ExternalOutput")
    ov_d = nc.dram_tensor("ov", [max(1, n_ov) * MAXL, H, BLK], f32,
                          kind="ExternalOutput")

    with tile.TileContext(nc) as tc:
        with (
            tc.tile_pool(name="consts", bufs=1) as consts,
            tc.tile_pool(name="slabs", bufs=10) as slabs,
            tc.tile_pool(name="psum", bufs=2, space="PSUM") as psump,
            tc.tile_pool(name="sig", bufs=4) as sigp,
            tc.tile_pool(name="gt", bufs=3) as gtp,
            tc.tile_pool(name="tc_", bufs=3) as tcp,
            tc.tile_pool(name="tmp", bufs=4) as tmpp,
            tc.tile_pool(name="state", bufs=8) as statep,
            tc.tile_pool(name="ovst", bufs=3) as ovstp,
        ):
            gtab = consts.tile([128, GATE4], bf16, tag="gtab")
            whha = consts.tile([128, GATE4], bf16, tag="whha")
            whhb = consts.tile([128, GATE4], bf16, tag="whhb")
            nc.sync.dma_start(out=gtab[:], in_=gtab_d[:])
            nc.sync.dma_start(out=whha[:], in_=whha_d[:])
            nc.sync.dma_start(out=whhb[:], in_=whhb_d[:])

            loop_cm = tc.For_i(0, reps, 1) if reps > 1 else nullcontext()
            with loop_cm:
                gstate = {}
                slab_idx = 0
                for (g, t) in sched:
                    grp = groups[g]
                    a, b = blocks[grp["a"]], blocks[grp["b"]]
                    La, Lb = a["L"], b["L"]

                    # gpsimd wants 32-aligned partition ranges: zero [96:128],
                    # the DMA then overwrites rows 96:100 with real one-hot data.
                    sA = slabs.tile([128, BLK], bf16, tag="slab", name="sA")
                    nc.gpsimd.memset(sA[96:128, :], 0.0)
                    if do_dma:
                        nc.sync.dma_start(out=sA[0:V, :], in_=oh_d[slab_idx])
                    slab_idx += 1
                    if t < Lb or not SKIPB:
                        sB = slabs.tile([128, BLK], bf16, tag="slab", name="sB")
                        nc.gpsimd.memset(sB[96:128, :], 0.0)
                        if do_dma:
                            nc.sync.dma_start(out=sB[0:V, :], in_=oh_d[slab_idx])
                    slab_idx += 1

                    ps = psump.tile([128, 4 * BLK], f32, tag="ps")
                    st = gstate.get(g)

                    # All matmuls use K=128 (vocab zero-padded; whhA/whhB have a
                    # zero half so block A/B recurrences pick out their own h).
                    # Per bank, A's accumulation group fully precedes B's: B's
                    # start=True clears the bank's has_written bits, which is only
                    # safe once A's group is complete.
                    if do_mm:
                        for q in range(4):
                            qs = slice(64 * q, 64 * q + 64)
                            cs = slice(BLK * q, BLK * q + BLK)
                            oA = ps[0:64, cs]
                            nc.tensor.matmul(oA, gtab[:, qs], sA[:, :],
                                             start=True, stop=(t == 0),
                                             tile_position=(0, 0))
                            if t > 0:
                                nc.tensor.matmul(oA, whha[:, qs], st["hb"][:, :],
                                                 start=False, stop=True,
                                                 tile_position=(0, 0))
                            if t < Lb or not SKIPB:
                                oB = ps[64:128, cs]
                                nc.tensor.matmul(oB, gtab[:, qs], sB[:, :],
                                                 start=True, stop=(t == 0),
                                                 tile_position=(0, 64))
                                if t > 0:
                                    nc.tensor.matmul(oB, whhb[:, qs],
                                                     st["hb"][:, :],
                                                     start=False, stop=True,
                                                     tile_position=(0, 64))

                    if one_mm:
                        nc.tensor.matmul(ps[0:64, 0:BLK], gtab[:, 0:64], sA[:, :],
                                         start=True, stop=True,
                                         tile_position=(0, 0))
                    if t == 0:
                        st = gstate[g] = {
                            "hb": statep.tile([128, BLK], bf16, tag="hb",
                                              name="hb"),
                            "c": statep.tile([128, BLK], f32, tag="c", name="c"),
                        }
                        if not do_act:
                            nc.gpsimd.memset(st["hb"][:, :], 0.0)
                            nc.gpsimd.memset(st["c"][:, :], 0.0)

                    cap_halves = [
                        (blk, half) for blk, half in
                        ((a, slice(0, 64)), (b, slice(64, 128)))
                        if blk["is_ov"] and t in blk.get("cap_steps", ())
                    ]
                    need_f32_h = (t == La - 1) or (t == Lb - 1) or bool(cap_halves)
                    # Once the shorter block B is finished, restrict the whole
                    # chain to A's partitions (same column cost, but avoids
                    # reading PSUM regions that were never written this step).
                    sl = slice(0, 128 if (t < Lb or not SKIPB) else 64)
                    if do_act:
                        sig = sigp.tile([128, 3 * BLK], f32, tag="sig")
                        nc.scalar.activation(out=sig[sl, :], in_=ps[sl, 0:3 * BLK],
                                             func=mybir.ActivationFunctionType.Sigmoid)
                        gt = gtp.tile([128, BLK], f32, tag="gt")
                        nc.scalar.activation(out=gt[sl, :], in_=ps[sl, 3 * BLK:4 * BLK],
                                             func=mybir.ActivationFunctionType.Tanh)

                        if t == 0:
                            nc.vector.tensor_mul(st["c"][sl, :], sig[sl, 0:BLK],
                                                 gt[sl, :])
                        else:
                            t1 = tmpp.tile([128, BLK], f32, tag="t1")
                            t2 = tmpp.tile([128, BLK], f32, tag="t2")
                            nc.vector.tensor_mul(t1[sl, :], sig[sl, 0:BLK],
                                                 gt[sl, :])
                            nc.vector.tensor_mul(t2[sl, :], sig[sl, BLK:2 * BLK],
                                                 st["c"][sl, :])
                            nc.vector.tensor_add(st["c"][sl, :], t1[sl, :],
                                                 t2[sl, :])

                        tch = tcp.tile([128, BLK], f32, tag="tc")
                        nc.scalar.activation(out=tch[sl, :], in_=st["c"][sl, :],
                                             func=mybir.ActivationFunctionType.Tanh)
                        # bf16 h feeds the next step's matmuls; a full-precision
                        # product is formed only when a block's output is due.
                        nc.vector.tensor_mul(st["hb"][sl, :],
                                             sig[sl, 2 * BLK:3 * BLK],
                                             tch[sl, :])
                        if need_f32_h:
                            hf = tmpp.tile([128, BLK], f32, tag="hf", name="hf")
                            nc.vector.tensor_mul(hf[sl, :],
                                                 sig[sl, 2 * BLK:3 * BLK],
                                                 tch[sl, :])
                    if not do_act:
                        need_f32_h = False

                    if need_f32_h:
                        if t == La - 1:
                            nc.sync.dma_start(out=out_d[grp["a"]],
                                              in_=hf[0:64, :])
                        if t == Lb - 1:
                            nc.sync.dma_start(out=out_d[grp["b"]],
                                              in_=hf[64:128, :])
                        for blk, half in cap_halves:
                            stg = ovstp.tile([64, BLK], f32, tag="ovst")
                            nc.vector.tensor_copy(stg, hf[half, :])
                            nc.sync.dma_start(
                                out=ov_d[blk["ov_idx"] * MAXL + t],
                                in_=stg[:])

    nc.compile()
    return nc


# --------------------------------------------------------------------------
# Entry point
# --------------------------------------------------------------------------

def kernel(emb, W_ih, W_hh, b_ih, b_hh, chars, lengths):
    from concourse.bass_utils import run_bass_kernel_spmd

    emb = np.asarray(emb, dtype=np.float32)
    W_ih = np.asarray(W_ih, dtype=np.float32)
    W_hh = np.asarray(W_hh, dtype=np.float32)
    b_ih = np.asarray(b_ih, dtype=np.float32)
    b_hh = np.asarray(b_hh, dtype=np.float32)
    chars = np.asarray(chars)
    lengths_np = np.asarray(lengths)

    n = chars.shape[0]

    # --- weight prep -------------------------------------------------------
    G = emb @ W_ih.T + b_ih + b_hh                      # [V, 4H]
    G = G[:, _GATE_PERM]
    gtab = np.zeros((128, GATE4), dtype=BF16)
    gtab[:V] = G.astype(BF16)
    whhT = W_hh.T[:, _GATE_PERM].astype(BF16)           # [H, 4H]
    zero = np.zeros_like(whhT)
    whhA = np.concatenate([whhT, zero], axis=0)         # [128, 4H]
    whhB = np.concatenate([zero, whhT], axis=0)         # [128, 4H]

    # --- word assignment ---------------------------------------------------
    plan = _plan(lengths_np)
    blocks, groups, sched = plan["blocks"], plan["groups"], plan["sched"]

    sig = (tuple((b["L"], b["is_ov"], b.get("cap_steps", ())) for b in blocks),
           tuple(sched))
    key = hash(sig)
    if key not in _PROGRAM_CACHE:
        _PROGRAM_CACHE[key] = _build_program(sig, blocks, groups, sched,
                                             plan["n_ov"])
    nc = _PROGRAM_CACHE[key]

    ohs = _build_onehots(plan, chars, lengths_np)
    in_maps = [{"oh": ohs[c], "gtab": gtab, "whha": whhA, "whhb": whhB}
               for c in range(NCORES)]

    res = run_bass_kernel_spmd(nc, in_maps, core_ids=list(range(NCORES)))
    kernel._last_nc = nc
    kernel._last_in_maps = in_maps

    # --- gather results ----------------------------------------------------
    outs = np.stack([r["out"] for r in res.results])    # [8, nb, H, BLK]
    ovs = np.stack([r["ov"] for r in res.results])      # [8, n_ov*16, H, BLK]

    result = np.empty((n, H), dtype=np.float32)
    for c in range(NCORES):
        for bi, blk in enumerate(blocks):
            words = plan["assign"][c][bi]
            valid = words >= 0
            if not valid.any():
                continue
            w = words[valid]
            cols = np.nonzero(valid)[0]
            if blk["is_ov"]:
                steps = lengths_np[w].astype(np.int64) - 1
                result[w] = ovs[c, blk["ov_idx"] * MAXL + steps, :, cols]
            else:
                result[w] = outs[c, bi, :, cols]
    return result



# revision 23
# speedup vs baseline: 3.1628x; 3.1628x over previous
"""Char-LSTM kernel for Trainium2 (8 NeuronCores, data parallel).

Strategy
--------
Host side:
  * Gather x_t = emb[chars[:, t]] per word per step on the host and ship
    [x_t; 1] slabs of shape [33, 1024] bf16 per scheduled entry (A half in
    columns 0:512, B half in 512:1024).  The device runs ONE fused matmul
    per gate bank with K = 97: stationary rows 0:64 = W_hh.T,
    64:96 = W_ih.T, row 96 = b_ih + b_hh.
  * Sort words by length into per-core blocks of 512 words of a single
    length each.  Lengths with more than 4096 words spill their excess into
    free (dummy) slots of SHORTER blocks; those words take the block-end
    (h, c) — h from the block's normal output, c from a one-off c-dump DMA —
    and the host finishes their remaining few steps in numpy.
  * Blocks pair into groups of 1024 words: block A on SBUF partitions 0:64,
    block B on 64:128 (state stored transposed, [H, words]).  Pairs are
    (16,15), (14,13), ..., (2,1); the pair runs min(La,Lb) shared steps, and
    the A blocks' single leftover final steps are themselves paired into 4
    combined "tail" entries: (16@15 | 14@13), (12@11 | 10@9), (8@7 | 6@5),
    (4@3 | 2@1).  Total entries = sum(len)/2 = 68 — each with all 128
    partitions active, which is what the Scalar (ACT) engine cost scales
    with.

Device side (identical SPMD program on all 8 cores), per entry:
  * 8 matmuls (4 gate banks x {A, B}), M=64, K=97, N=512, col-packed into
    the PE array via tile_position (0,0)/(0,64), accumulating into one
    [128, 2048] PSUM tile laid out as banks [i | f | o | g].
  * One sigmoid over [128, 1536] (i,f,o) -> bf16, tanh over g -> bf16,
    then the cell update on the Vector engine in bf16 (2x DVE mode),
    tanh(c) and h = o * tanh(c) written into the stream tile of the block's
    next entry.
  Entries of independent chains interleave so the recurrence pipelines
  across the Tensor/Scalar/Vector engines.
"""

import os
import sys

for _p in ("/opt/trn_rl_repo", "/root/.axon_site/_ro/trn_rl_repo"):
    if os.path.isdir(_p) and _p not in sys.path:
        sys.path.insert(0, _p)

import numpy as np
import ml_dtypes

BF16 = ml_dtypes.bfloat16

H = 64          # hidden size
E = 32          # char embedding size
V = 100         # vocab
MAXL = 16       # max word length
BLK = 512       # words per block (one half of an entry)
NCORES = 8
GATE4 = 4 * H   # 256
KDIM = H + E + 1  # 97: [h; x; 1] contraction

# torch gate order in the weights is [i, f, g, o]; we stage banks as
# [i, f, o, g] so sigmoid covers one contiguous [128, 1536] span.
_GATE_PERM = np.concatenate([
    np.arange(0, 64),        # i
    np.arange(64, 128),      # f
    np.arange(192, 256),     # o
    np.arange(128, 192),     # g
])

INTERLEAVE = int(os.environ.get("LSTM_INTERLEAVE", "8"))
_PROGRAM_CACHE = {}


# --------------------------------------------------------------------------
# Host-side planning
# --------------------------------------------------------------------------

def _plan(lengths):
    """Assign words to (core, block, column) slots and build the entry list.

    blocks[i] has length 16-i (i = 0..15).  Entry = ((bi_a, ta), (bi_b, tb)).

    Returns dict with blocks, entries, assign, foreign (host-finish list).
    """
    lengths = lengths.astype(np.int64)
    order = np.argsort(lengths, kind="stable")

    per_core_words = {}   # (core, L) -> [BLK] array
    leftovers = []        # (word, length) spills
    for L in range(1, MAXL + 1):
        idx = order[np.searchsorted(lengths, L, side="left", sorter=order):
                    np.searchsorted(lengths, L, side="right", sorter=order)]
        take = idx[: NCORES * BLK]
        if idx.shape[0] > NCORES * BLK:
            leftovers.extend((int(w), L) for w in idx[NCORES * BLK:])
        arr = np.full(NCORES * BLK, -1, dtype=np.int64)
        arr[: take.shape[0]] = take
        arr = arr.reshape(NCORES, BLK)
        for c in range(NCORES):
            per_core_words[(c, L)] = arr[c].copy()

    block_lens = list(range(MAXL, 0, -1))       # [16, 15, ..., 1]
    bi_of_len = {L: i for i, L in enumerate(block_lens)}

    # place leftovers into free slots of blocks with L' <= word length,
    # preferring the longest L' (fewest host steps)
    foreign = []   # (word, core, block_idx, col, L_placed)
    dump_c = set()
    free = []
    for Lp in range(MAXL, 0, -1):
        for c in range(NCORES):
            arr = per_core_words[(c, Lp)]
            for col in np.nonzero(arr < 0)[0]:
                free.append((Lp, c, int(col)))
    leftovers.sort(key=lambda wl: -wl[1])
    fptr = 0
    for w, L in leftovers:
        while fptr < len(free) and free[fptr][0] > L:
            fptr += 1
        if fptr >= len(free):
            # no block slot fits: full host compute from (h, c) = 0
            foreign.append((w, 0, 0, 0, 0))
            continue
        Lp, c, col = free[fptr]
        fptr += 1
        per_core_words[(c, Lp)][col] = w
        foreign.append((w, c, bi_of_len[Lp], col, Lp))
        dump_c.add(bi_of_len[Lp])

    blocks = [{"L": L, "dump_c": (bi_of_len[L] in dump_c)}
              for L in block_lens]
    assign = [[per_core_words[(c, blocks[i]["L"])] for i in range(MAXL)]
              for c in range(NCORES)]

    # --- entry schedule ---------------------------------------------------
    # groups g = (block 2g, block 2g+1), La = 16-2g, Lb = La-1; each group is
    # a chain of Lb shared entries.  The 4 tail entries are 1-entry chains
    # gated on both parent groups.  Schedule with a fixed INTERLEAVE-slot
    # rotation, refilling empty slots with the longest ready chain, so chain
    # ends stagger and the schedule's end keeps independent work.
    n_groups = MAXL // 2
    chains = []   # (entries_list, deps: set of chain ids)
    for g in range(n_groups):
        Lb = blocks[2 * g + 1]["L"]
        chains.append(([((2 * g, t), (2 * g + 1, t)) for t in range(Lb)],
                       set()))
    for j in range(4):
        ba, bb = 4 * j, 4 * j + 2
        chains.append(([((ba, blocks[ba]["L"] - 1),
                         (bb, blocks[bb]["L"] - 1))],
                       {2 * j, 2 * j + 1}))
    # Hand-packed 3-slot rotation (chain lengths are fixed: groups
    # 15,13,11,9,7,5,3,1 entries + four 1-entry tails T0..T3 = 68).
    # Every chain's consecutive entries are >= 3 apart and every tail runs
    # >= 3 entries after both parents' last entries; all 3 slots stay busy
    # through round 21 of 23 (68 = 3*22 + 2).
    T0, T1, T2, T3 = 8, 9, 10, 11
    slots = [[0, 5, T3, T0, T2],     # g0(15) g5(5)  tails
             [1, 3, T1],             # g1(13) g3(9)
             [7, 6, 4, 2]]           # g7(1)  g6(3) g4(7) g2(11)
    assert [len(c[0]) for c in chains] == [15, 13, 11, 9, 7, 5, 3, 1,
                                           1, 1, 1, 1]
    queues = [[e for cid in s for e in chains[cid][0]] for s in slots]
    entries = []
    while any(queues):
        for q in queues:
            if q:
                entries.append(q.pop(0))

    return {"blocks": blocks, "entries": entries,
            "assign": assign, "foreign": foreign}


def _build_xslabs(plan, chars, lengths, emb_bf):
    """Per-core x slab tensors [n_entries, E+1, 2*BLK] bf16: rows 0:32 hold
    emb[chars[:, t]].T (A half cols 0:512, B half 512:1024), row 32 ones."""
    blocks, entries = plan["blocks"], plan["entries"]
    out = []
    for c in range(NCORES):
        xs = np.zeros((len(entries), E + 1, 2 * BLK), dtype=BF16)
        xs[:, E, :] = BF16(1.0)
        for k, entry in enumerate(entries):
            for hi, (b, t) in enumerate(entry):
                words = plan["assign"][c][b]
                valid = (words >= 0)
                w = words[valid]
                if w.shape[0]:
                    alive = t < lengths[w]
                    cols = np.nonzero(valid)[0][alive] + hi * BLK
                    ch = chars[w[alive], t]
                    xs[k, :E, cols] = emb_bf[ch]
        out.append(xs)
    return out


# --------------------------------------------------------------------------
# Device program
# --------------------------------------------------------------------------

def _build_program(plan_sig, blocks, entries, reps=1):
    import concourse.bass as bass
    import concourse.tile as tile
    from concourse import bacc, mybir
    from contextlib import nullcontext

    f32 = mybir.dt.float32
    bf16 = mybir.dt.bfloat16
    n_blocks = len(blocks)
    n_entries = len(entries)
    n_groups = n_blocks // 2

    # (block, t) -> (entry_idx, half)
    where = {}
    for k, entry in enumerate(entries):
        for hi, (b, t) in enumerate(entry):
            where[(b, t)] = (k, hi)
    # entry kind: shared entries of group g have (2g, t), (2g+1, t);
    # tails have bb == ba + 2.
    def entry_group(entry):
        (ba, ta), (bb, tb) = entry
        return ba // 2 if bb == ba + 1 else None

    nc = bacc.Bacc("TRN2", target_bir_lowering=False, debug=False,
                   num_devices=NCORES)
    xs_d = nc.dram_tensor("xs", [n_entries, E + 1, 2 * BLK], bf16,
                          kind="ExternalInput")
    wc_d = nc.dram_tensor("wc", [128, GATE4], bf16, kind="ExternalInput")
    out_d = nc.dram_tensor("out", [n_blocks, H, BLK], bf16,
                           kind="ExternalOutput")
    cv_d = nc.dram_tensor("cv", [n_blocks, H, BLK], bf16,
                          kind="ExternalOutput")

    with tile.TileContext(nc) as tc:
        with (
            tc.tile_pool(name="consts", bufs=1) as consts,
            tc.tile_pool(name="xh", bufs=22) as xhp,
            tc.tile_pool(name="xtail", bufs=4) as xtailp,
            tc.tile_pool(name="psum", bufs=2, space="PSUM") as psump,
            tc.tile_pool(name="sig", bufs=10) as sigp,
            tc.tile_pool(name="gt", bufs=8) as gtp,
            tc.tile_pool(name="tc_", bufs=8) as tcp,
            tc.tile_pool(name="tmp", bufs=10) as tmpp,
            tc.tile_pool(name="state", bufs=14) as statep,
        ):
            wc = consts.tile([128, GATE4], bf16, tag="wc")
            nc.sync.dma_start(out=wc[:], in_=wc_d[:])

            loop_cm = tc.For_i(0, reps, 1) if reps > 1 else nullcontext()
            with loop_cm:
                ent_tile = {}
                tail_ids = {k for k, e in enumerate(entries)
                            if e[1][0] != e[0][0] + 1}

                def get_tile(k):
                    if k not in ent_tile:
                        # tail tiles are written by parents that finish much
                        # earlier -> long lifetime -> dedicated pool
                        pool = xtailp if k in tail_ids else xhp
                        tl = pool.tile([128, 2 * BLK], bf16, tag="xh",
                                       name=f"x{k}")
                        nc.sync.dma_start(out=tl[H:KDIM, :], in_=xs_d[k])
                        ent_tile[k] = tl
                    return ent_tile[k]

                gc = {}      # group -> c tile

                for k, entry in enumerate(entries):
                    (ba, ta), (bb, tb) = entry
                    La, Lb = blocks[ba]["L"], blocks[bb]["L"]
                    g = entry_group(entry)
                    is_tail = g is None

                    xab = get_tile(k)
                    if g is not None and ta == 0:
                        nc.gpsimd.memset(xab[0:H, :], 0.0)
                        gc[g] = statep.tile([128, BLK], bf16, tag="c",
                                            name=f"c{g}")

                    ps = psump.tile([128, 4 * BLK], f32, tag="ps")
                    for q in range(4):
                        qs = slice(64 * q, 64 * q + 64)
                        cs = slice(BLK * q, BLK * q + BLK)
                        nc.tensor.matmul(ps[0:64, cs], wc[0:KDIM, qs],
                                         xab[0:KDIM, 0:BLK], start=True,
                                         stop=True, tile_position=(0, 0))
                        nc.tensor.matmul(ps[64:128, cs], wc[0:KDIM, qs],
                                         xab[0:KDIM, BLK:2 * BLK],
                                         start=True, stop=True,
                                         tile_position=(0, 64))

                    sig = sigp.tile([128, 3 * BLK], bf16, tag="sig")
                    nc.scalar.activation(out=sig[:, :], in_=ps[:, 0:3 * BLK],
                                         func=mybir.ActivationFunctionType.Sigmoid)
                    gt = gtp.tile([128, BLK], bf16, tag="gt")
                    nc.scalar.activation(out=gt[:, :], in_=ps[:, 3 * BLK:4 * BLK],
                                         func=mybir.ActivationFunctionType.Tanh)

                    # cell update
                    if is_tail:
                        # assemble [cA | cB] from the parents' group c tiles
                        # (A halves of groups ba//2 and bb//2)
                        ct = statep.tile([128, BLK], bf16, tag="c",
                                         name=f"ct{k}")
                        nc.vector.tensor_copy(ct[0:64, :],
                                              gc[ba // 2][0:64, :])
                        nc.vector.tensor_copy(ct[64:128, :],
                                              gc[bb // 2][0:64, :])
                    else:
                        ct = gc[g]

                    if not is_tail and ta == 0:
                        nc.vector.tensor_mul(ct[:, :], sig[:, 0:BLK], gt[:, :])
                    else:
                        t1 = tmpp.tile([128, BLK], bf16, tag="t1")
                        t2 = tmpp.tile([128, BLK], bf16, tag="t2")
                        nc.vector.tensor_mul(t2[:, :], sig[:, BLK:2 * BLK],
                                             ct[:, :])
                        nc.vector.tensor_mul(t1[:, :], sig[:, 0:BLK], gt[:, :])
                        nc.vector.tensor_add(ct[:, :], t1[:, :], t2[:, :])

                    tch = tcp.tile([128, BLK], bf16, tag="tc")
                    nc.scalar.activation(out=tch[:, :], in_=ct[:, :],
                                         func=mybir.ActivationFunctionType.Tanh)

                    # h = o * tanh(c): route each half to its next entry's
                    # stream tile, or to hf + output DMA at a block's end.
                    final = []
                    for hi, (b, t) in enumerate(entry):
                        pr = slice(64 * hi, 64 * hi + 64)
                        nxt = where.get((b, t + 1))
                        if nxt is not None:
                            nk, nhi = nxt
                            dst = get_tile(nk)[0:H, nhi * BLK:(nhi + 1) * BLK]
                            nc.vector.tensor_mul(dst, sig[pr, 2 * BLK:3 * BLK],
                                                 tch[pr, :])
                        else:
                            final.append((hi, b))
                    if final:
                        hf = tmpp.tile([128, BLK], bf16, tag="hf", name="hf")
                        lo = min(hi for hi, _ in final) * 64
                        hi_ = max(hi for hi, _ in final) * 64 + 64
                        nc.vector.tensor_mul(hf[lo:hi_, :],
                                             sig[lo:hi_, 2 * BLK:3 * BLK],
                                             tch[lo:hi_, :])
                        for hi, b in final:
                            pr = slice(64 * hi, 64 * hi + 64)
                            nc.sync.dma_start(out=out_d[b], in_=hf[pr, :])
                            if blocks[b]["dump_c"]:
                                nc.sync.dma_start(out=cv_d[b], in_=ct[pr, :])

    nc.compile()
    return nc


# --------------------------------------------------------------------------
# Entry point
# --------------------------------------------------------------------------

def _host_finish(foreign, outs, cvs, emb, W_ih, W_hh, b_ih, b_hh, chars,
                 lengths, result):
    """Finish leftover words on the host: from the block-end (h, c) at step
    L_placed, run the remaining steps in fp32 numpy."""
    if not foreign:
        return
    W = np.array([f[0] for f in foreign])
    core = np.array([f[1] for f in foreign])
    bi = np.array([f[2] for f in foreign])
    col = np.array([f[3] for f in foreign])
    start = np.array([f[4] for f in foreign])
    h = outs[core, bi, :, col].astype(np.float32)     # [n, H]
    c = cvs[core, bi, :, col].astype(np.float32)
    h[start == 0] = 0.0
    c[start == 0] = 0.0
    target = lengths[W]
    bias = (b_ih + b_hh)[None, :]
    for t in range(int(start.min()), int(target.max())):
        act = (start <= t) & (t < target)
        if not act.any():
            continue
        x = emb[chars[W[act], t]]                     # [m, E]
        gates = x @ W_ih.T + h[act] @ W_hh.T + bias   # [m, 4H] torch i,f,g,o
        ig = 1.0 / (1.0 + np.exp(-gates[:, 0:H]))
        fg = 1.0 / (1.0 + np.exp(-gates[:, H:2 * H]))
        gg = np.tanh(gates[:, 2 * H:3 * H])
        og = 1.0 / (1.0 + np.exp(-gates[:, 3 * H:4 * H]))
        cn = fg * c[act] + ig * gg
        h[act] = og * np.tanh(cn)
        c[act] = cn
    result[W] = h


def kernel(emb, W_ih, W_hh, b_ih, b_hh, chars, lengths):
    from concourse.bass_utils import run_bass_kernel_spmd

    emb = np.asarray(emb, dtype=np.float32)
    W_ih = np.asarray(W_ih, dtype=np.float32)
    W_hh = np.asarray(W_hh, dtype=np.float32)
    b_ih = np.asarray(b_ih, dtype=np.float32)
    b_hh = np.asarray(b_hh, dtype=np.float32)
    chars = np.asarray(chars)
    lengths_np = np.asarray(lengths).astype(np.int64)

    n = chars.shape[0]

    # --- weight prep -------------------------------------------------------
    wc = np.zeros((128, GATE4), dtype=BF16)
    wc[0:H] = W_hh.T[:, _GATE_PERM].astype(BF16)
    wc[H:H + E] = W_ih.T[:, _GATE_PERM].astype(BF16)
    wc[H + E] = (b_ih + b_hh)[_GATE_PERM].astype(BF16)
    emb_bf = emb.astype(BF16)

    # --- word assignment ---------------------------------------------------
    plan = _plan(lengths_np)
    blocks, entries = plan["blocks"], plan["entries"]

    sig = (tuple((b["L"], b["dump_c"]) for b in blocks), tuple(entries))
    key = hash(sig)
    if key not in _PROGRAM_CACHE:
        _PROGRAM_CACHE[key] = _build_program(sig, blocks, entries)
    nc = _PROGRAM_CACHE[key]

    xss = _build_xslabs(plan, chars, lengths_np, emb_bf)
    in_maps = [{"xs": xss[c], "wc": wc} for c in range(NCORES)]

    res = run_bass_kernel_spmd(nc, in_maps, core_ids=list(range(NCORES)))
    kernel._last_nc = nc
    kernel._last_in_maps = in_maps

    # --- gather results ----------------------------------------------------
    outs = np.stack([np.asarray(r["out"]) for r in res.results])
    cvs = np.stack([np.asarray(r["cv"]) for r in res.results])

    result = np.empty((n, H), dtype=np.float32)
    for c in range(NCORES):
        for bi in range(len(blocks)):
            words = plan["assign"][c][bi]
            valid = words >= 0
            if not valid.any():
                continue
            w = words[valid]
            cols = np.nonzero(valid)[0]
            result[w] = outs[c, bi, :, cols].astype(np.float32)
    _host_finish(plan["foreign"], outs, cvs, emb, W_ih, W_hh, b_ih, b_hh,
                 chars, lengths_np, result)
    return result


# revision 24
# speedup vs baseline: 3.1844x; 1.0068x over previous
"""Char-LSTM kernel for Trainium2 (8 NeuronCores, data parallel).

Strategy
--------
Host side:
  * Gather x_t = emb[chars[:, t]] per word per step on the host and ship
    [x_t; 1] slabs of shape [33, 1024] bf16 per scheduled entry (A half in
    columns 0:512, B half in 512:1024).  The device runs ONE fused matmul
    per gate bank with K = 97: stationary rows 0:64 = W_hh.T,
    64:96 = W_ih.T, row 96 = b_ih + b_hh.
  * Sort words by length into per-core blocks of 512 words of a single
    length each.  Lengths with more than 4096 words spill their excess into
    free (dummy) slots of SHORTER blocks; those words take the block-end
    (h, c) — h from the block's normal output, c from a one-off c-dump DMA —
    and the host finishes their remaining few steps in numpy.
  * Blocks pair into groups of 1024 words: block A on SBUF partitions 0:64,
    block B on 64:128 (state stored transposed, [H, words]).  Pairs are
    (16,15), (14,13), ..., (2,1); the pair runs min(La,Lb) shared steps, and
    the A blocks' single leftover final steps are themselves paired into 4
    combined "tail" entries: (16@15 | 14@13), (12@11 | 10@9), (8@7 | 6@5),
    (4@3 | 2@1).  Total entries = sum(len)/2 = 68 — each with all 128
    partitions active, which is what the Scalar (ACT) engine cost scales
    with.

Device side (identical SPMD program on all 8 cores), per entry:
  * 8 matmuls (4 gate banks x {A, B}), M=64, K=97, N=512, col-packed into
    the PE array via tile_position (0,0)/(0,64), accumulating into one
    [128, 2048] PSUM tile laid out as banks [i | f | o | g].
  * One sigmoid over [128, 1536] (i,f,o) -> bf16, tanh over g -> bf16,
    then the cell update on the Vector engine in bf16 (2x DVE mode),
    tanh(c) and h = o * tanh(c) written into the stream tile of the block's
    next entry.
  Entries of independent chains interleave so the recurrence pipelines
  across the Tensor/Scalar/Vector engines.
"""

import os
import sys

for _p in ("/opt/trn_rl_repo", "/root/.axon_site/_ro/trn_rl_repo"):
    if os.path.isdir(_p) and _p not in sys.path:
        sys.path.insert(0, _p)

import numpy as np
import ml_dtypes

BF16 = ml_dtypes.bfloat16

H = 64          # hidden size
E = 32          # char embedding size
V = 100         # vocab
MAXL = 16       # max word length
BLK = 512       # words per block (one half of an entry)
NCORES = 8
GATE4 = 4 * H   # 256
KDIM = H + E + 1  # 97: [h; x; 1] contraction

# torch gate order in the weights is [i, f, g, o]; we stage banks as
# [i, f, o, g] so sigmoid covers one contiguous [128, 1536] span.
_GATE_PERM = np.concatenate([
    np.arange(0, 64),        # i
    np.arange(64, 128),      # f
    np.arange(192, 256),     # o
    np.arange(128, 192),     # g
])

INTERLEAVE = int(os.environ.get("LSTM_INTERLEAVE", "8"))
_PROGRAM_CACHE = {}


# --------------------------------------------------------------------------
# Host-side planning
# --------------------------------------------------------------------------

def _plan(lengths):
    """Assign words to (core, block, column) slots and build the entry list.

    blocks[i] has length 16-i (i = 0..15).  Entry = ((bi_a, ta), (bi_b, tb)).

    Returns dict with blocks, entries, assign, foreign (host-finish list).
    """
    lengths = lengths.astype(np.int64)
    order = np.argsort(lengths, kind="stable")

    per_core_words = {}   # (core, L) -> [BLK] array
    leftovers = []        # (word, length) spills
    for L in range(1, MAXL + 1):
        idx = order[np.searchsorted(lengths, L, side="left", sorter=order):
                    np.searchsorted(lengths, L, side="right", sorter=order)]
        take = idx[: NCORES * BLK]
        if idx.shape[0] > NCORES * BLK:
            leftovers.extend((int(w), L) for w in idx[NCORES * BLK:])
        arr = np.full(NCORES * BLK, -1, dtype=np.int64)
        arr[: take.shape[0]] = take
        arr = arr.reshape(NCORES, BLK)
        for c in range(NCORES):
            per_core_words[(c, L)] = arr[c].copy()

    block_lens = list(range(MAXL, 0, -1))       # [16, 15, ..., 1]
    bi_of_len = {L: i for i, L in enumerate(block_lens)}

    # place leftovers into free slots of blocks with L' <= word length,
    # preferring the longest L' (fewest host steps)
    foreign = []   # (word, core, block_idx, col, L_placed)
    dump_c = set()
    free = []
    for Lp in range(MAXL, 0, -1):
        for c in range(NCORES):
            arr = per_core_words[(c, Lp)]
            for col in np.nonzero(arr < 0)[0]:
                free.append((Lp, c, int(col)))
    leftovers.sort(key=lambda wl: -wl[1])
    fptr = 0
    for w, L in leftovers:
        while fptr < len(free) and free[fptr][0] > L:
            fptr += 1
        if fptr >= len(free):
            # no block slot fits: full host compute from (h, c) = 0
            foreign.append((w, 0, 0, 0, 0))
            continue
        Lp, c, col = free[fptr]
        fptr += 1
        per_core_words[(c, Lp)][col] = w
        foreign.append((w, c, bi_of_len[Lp], col, Lp))
        dump_c.add(bi_of_len[Lp])

    blocks = [{"L": L, "dump_c": (bi_of_len[L] in dump_c)}
              for L in block_lens]
    assign = [[per_core_words[(c, blocks[i]["L"])] for i in range(MAXL)]
              for c in range(NCORES)]

    # --- entry schedule ---------------------------------------------------
    # groups g = (block 2g, block 2g+1), La = 16-2g, Lb = La-1; each group is
    # a chain of Lb shared entries.  The 4 tail entries are 1-entry chains
    # gated on both parent groups.  Schedule with a fixed INTERLEAVE-slot
    # rotation, refilling empty slots with the longest ready chain, so chain
    # ends stagger and the schedule's end keeps independent work.
    n_groups = MAXL // 2
    chains = []   # (entries_list, deps: set of chain ids)
    for g in range(n_groups):
        Lb = blocks[2 * g + 1]["L"]
        chains.append(([((2 * g, t), (2 * g + 1, t)) for t in range(Lb)],
                       set()))
    for j in range(4):
        ba, bb = 4 * j, 4 * j + 2
        chains.append(([((ba, blocks[ba]["L"] - 1),
                         (bb, blocks[bb]["L"] - 1))],
                       {2 * j, 2 * j + 1}))
    # Hand-packed 4-slot rotation (chain lengths are fixed: groups
    # 15,13,11,9,7,5,3,1 entries + four 1-entry tails T0..T3 = 68 = 4*17).
    # Perfectly balanced: every slot runs 17 rounds, every chain's
    # consecutive entries are >= 4 apart, every tail runs >= 4 entries
    # after both parents, and the final round is 4 independent tails.
    T0, T1, T2, T3 = 8, 9, 10, 11
    slots = [[0, 7, T0],             # g0(15) g7(1)
             [1, 6, T3],             # g1(13) g6(3)
             [2, 5, T1],             # g2(11) g5(5)
             [3, 4, T2]]             # g3(9)  g4(7)
    assert [len(c[0]) for c in chains] == [15, 13, 11, 9, 7, 5, 3, 1,
                                           1, 1, 1, 1]
    queues = [[e for cid in s for e in chains[cid][0]] for s in slots]
    entries = []
    while any(queues):
        for q in queues:
            if q:
                entries.append(q.pop(0))

    return {"blocks": blocks, "entries": entries,
            "assign": assign, "foreign": foreign}


def _build_xslabs(plan, chars, lengths, emb_bf):
    """Per-core x slab tensors [n_entries, E+1, 2*BLK] bf16: rows 0:32 hold
    emb[chars[:, t]].T (A half cols 0:512, B half 512:1024), row 32 ones."""
    blocks, entries = plan["blocks"], plan["entries"]
    out = []
    for c in range(NCORES):
        xs = np.zeros((len(entries), E + 1, 2 * BLK), dtype=BF16)
        xs[:, E, :] = BF16(1.0)
        for k, entry in enumerate(entries):
            for hi, (b, t) in enumerate(entry):
                words = plan["assign"][c][b]
                valid = (words >= 0)
                w = words[valid]
                if w.shape[0]:
                    alive = t < lengths[w]
                    cols = np.nonzero(valid)[0][alive] + hi * BLK
                    ch = chars[w[alive], t]
                    xs[k, :E, cols] = emb_bf[ch]
        out.append(xs)
    return out


# --------------------------------------------------------------------------
# Device program
# --------------------------------------------------------------------------

def _build_program(plan_sig, blocks, entries, reps=1):
    import concourse.bass as bass
    import concourse.tile as tile
    from concourse import bacc, mybir
    from contextlib import nullcontext

    f32 = mybir.dt.float32
    bf16 = mybir.dt.bfloat16
    n_blocks = len(blocks)
    n_entries = len(entries)
    n_groups = n_blocks // 2

    # (block, t) -> (entry_idx, half)
    where = {}
    for k, entry in enumerate(entries):
        for hi, (b, t) in enumerate(entry):
            where[(b, t)] = (k, hi)
    # entry kind: shared entries of group g have (2g, t), (2g+1, t);
    # tails have bb == ba + 2.
    def entry_group(entry):
        (ba, ta), (bb, tb) = entry
        return ba // 2 if bb == ba + 1 else None

    nc = bacc.Bacc("TRN2", target_bir_lowering=False, debug=False,
                   num_devices=NCORES)
    xs_d = nc.dram_tensor("xs", [n_entries, E + 1, 2 * BLK], bf16,
                          kind="ExternalInput")
    wc_d = nc.dram_tensor("wc", [128, GATE4], bf16, kind="ExternalInput")
    out_d = nc.dram_tensor("out", [n_blocks, H, BLK], bf16,
                           kind="ExternalOutput")
    cv_d = nc.dram_tensor("cv", [n_blocks, H, BLK], bf16,
                          kind="ExternalOutput")

    with tile.TileContext(nc) as tc:
        with (
            tc.tile_pool(name="consts", bufs=1) as consts,
            tc.tile_pool(name="xh", bufs=22) as xhp,
            tc.tile_pool(name="xtail", bufs=4) as xtailp,
            tc.tile_pool(name="psum", bufs=2, space="PSUM") as psump,
            tc.tile_pool(name="sig", bufs=10) as sigp,
            tc.tile_pool(name="gt", bufs=8) as gtp,
            tc.tile_pool(name="tc_", bufs=8) as tcp,
            tc.tile_pool(name="tmp", bufs=10) as tmpp,
            tc.tile_pool(name="state", bufs=14) as statep,
        ):
            wc = consts.tile([128, GATE4], bf16, tag="wc")
            nc.sync.dma_start(out=wc[:], in_=wc_d[:])

            loop_cm = tc.For_i(0, reps, 1) if reps > 1 else nullcontext()
            with loop_cm:
                ent_tile = {}
                tail_ids = {k for k, e in enumerate(entries)
                            if e[1][0] != e[0][0] + 1}

                def get_tile(k):
                    if k not in ent_tile:
                        # tail tiles are written by parents that finish much
                        # earlier -> long lifetime -> dedicated pool
                        pool = xtailp if k in tail_ids else xhp
                        tl = pool.tile([128, 2 * BLK], bf16, tag="xh",
                                       name=f"x{k}")
                        nc.sync.dma_start(out=tl[H:KDIM, :], in_=xs_d[k])
                        ent_tile[k] = tl
                    return ent_tile[k]

                gc = {}      # group -> c tile

                for k, entry in enumerate(entries):
                    (ba, ta), (bb, tb) = entry
                    La, Lb = blocks[ba]["L"], blocks[bb]["L"]
                    g = entry_group(entry)
                    is_tail = g is None

                    xab = get_tile(k)
                    if g is not None and ta == 0:
                        nc.gpsimd.memset(xab[0:H, :], 0.0)
                        gc[g] = statep.tile([128, BLK], bf16, tag="c",
                                            name=f"c{g}")

                    ps = psump.tile([128, 4 * BLK], f32, tag="ps")
                    for q in range(4):
                        qs = slice(64 * q, 64 * q + 64)
                        cs = slice(BLK * q, BLK * q + BLK)
                        nc.tensor.matmul(ps[0:64, cs], wc[0:KDIM, qs],
                                         xab[0:KDIM, 0:BLK], start=True,
                                         stop=True, tile_position=(0, 0))
                        nc.tensor.matmul(ps[64:128, cs], wc[0:KDIM, qs],
                                         xab[0:KDIM, BLK:2 * BLK],
                                         start=True, stop=True,
                                         tile_position=(0, 64))

                    sig = sigp.tile([128, 3 * BLK], bf16, tag="sig")
                    nc.scalar.activation(out=sig[:, :], in_=ps[:, 0:3 * BLK],
                                         func=mybir.ActivationFunctionType.Sigmoid)
                    gt = gtp.tile([128, BLK], bf16, tag="gt")
                    nc.scalar.activation(out=gt[:, :], in_=ps[:, 3 * BLK:4 * BLK],
                                         func=mybir.ActivationFunctionType.Tanh)

                    # cell update
                    if is_tail:
                        # assemble [cA | cB] from the parents' group c tiles
                        # (A halves of groups ba//2 and bb//2)
                        ct = statep.tile([128, BLK], bf16, tag="c",
                                         name=f"ct{k}")
                        nc.vector.tensor_copy(ct[0:64, :],
                                              gc[ba // 2][0:64, :])
                        nc.vector.tensor_copy(ct[64:128, :],
                                              gc[bb // 2][0:64, :])
                    else:
                        ct = gc[g]

                    if not is_tail and ta == 0:
                        nc.vector.tensor_mul(ct[:, :], sig[:, 0:BLK], gt[:, :])
                    else:
                        t1 = tmpp.tile([128, BLK], bf16, tag="t1")
                        t2 = tmpp.tile([128, BLK], bf16, tag="t2")
                        nc.vector.tensor_mul(t2[:, :], sig[:, BLK:2 * BLK],
                                             ct[:, :])
                        nc.vector.tensor_mul(t1[:, :], sig[:, 0:BLK], gt[:, :])
                        nc.vector.tensor_add(ct[:, :], t1[:, :], t2[:, :])

                    tch = tcp.tile([128, BLK], bf16, tag="tc")
                    nc.scalar.activation(out=tch[:, :], in_=ct[:, :],
                                         func=mybir.ActivationFunctionType.Tanh)

                    # h = o * tanh(c): route each half to its next entry's
                    # stream tile, or to hf + output DMA at a block's end.
                    final = []
                    for hi, (b, t) in enumerate(entry):
                        pr = slice(64 * hi, 64 * hi + 64)
                        nxt = where.get((b, t + 1))
                        if nxt is not None:
                            nk, nhi = nxt
                            dst = get_tile(nk)[0:H, nhi * BLK:(nhi + 1) * BLK]
                            nc.vector.tensor_mul(dst, sig[pr, 2 * BLK:3 * BLK],
                                                 tch[pr, :])
                        else:
                            final.append((hi, b))
                    if final:
                        hf = tmpp.tile([128, BLK], bf16, tag="hf", name="hf")
                        lo = min(hi for hi, _ in final) * 64
                        hi_ = max(hi for hi, _ in final) * 64 + 64
                        nc.vector.tensor_mul(hf[lo:hi_, :],
                                             sig[lo:hi_, 2 * BLK:3 * BLK],
                                             tch[lo:hi_, :])
                        for hi, b in final:
                            pr = slice(64 * hi, 64 * hi + 64)
                            nc.sync.dma_start(out=out_d[b], in_=hf[pr, :])
                            if blocks[b]["dump_c"]:
                                nc.sync.dma_start(out=cv_d[b], in_=ct[pr, :])

    nc.compile()
    return nc


# --------------------------------------------------------------------------
# Entry point
# --------------------------------------------------------------------------

def _host_finish(foreign, outs, cvs, emb, W_ih, W_hh, b_ih, b_hh, chars,
                 lengths, result):
    """Finish leftover words on the host: from the block-end (h, c) at step
    L_placed, run the remaining steps in fp32 numpy."""
    if not foreign:
        return
    W = np.array([f[0] for f in foreign])
    core = np.array([f[1] for f in foreign])
    bi = np.array([f[2] for f in foreign])
    col = np.array([f[3] for f in foreign])
    start = np.array([f[4] for f in foreign])
    h = outs[core, bi, :, col].astype(np.float32)     # [n, H]
    c = cvs[core, bi, :, col].astype(np.float32)
    h[start == 0] = 0.0
    c[start == 0] = 0.0
    target = lengths[W]
    bias = (b_ih + b_hh)[None, :]
    for t in range(int(start.min()), int(target.max())):
        act = (start <= t) & (t < target)
        if not act.any():
            continue
        x = emb[chars[W[act], t]]                     # [m, E]
        gates = x @ W_ih.T + h[act] @ W_hh.T + bias   # [m, 4H] torch i,f,g,o
        ig = 1.0 / (1.0 + np.exp(-gates[:, 0:H]))
        fg = 1.0 / (1.0 + np.exp(-gates[:, H:2 * H]))
        gg = np.tanh(gates[:, 2 * H:3 * H])
        og = 1.0 / (1.0 + np.exp(-gates[:, 3 * H:4 * H]))
        cn = fg * c[act] + ig * gg
        h[act] = og * np.tanh(cn)
        c[act] = cn
    result[W] = h


def kernel(emb, W_ih, W_hh, b_ih, b_hh, chars, lengths):
    from concourse.bass_utils import run_bass_kernel_spmd

    emb = np.asarray(emb, dtype=np.float32)
    W_ih = np.asarray(W_ih, dtype=np.float32)
    W_hh = np.asarray(W_hh, dtype=np.float32)
    b_ih = np.asarray(b_ih, dtype=np.float32)
    b_hh = np.asarray(b_hh, dtype=np.float32)
    chars = np.asarray(chars)
    lengths_np = np.asarray(lengths).astype(np.int64)

    n = chars.shape[0]

    # --- weight prep -------------------------------------------------------
    wc = np.zeros((128, GATE4), dtype=BF16)
    wc[0:H] = W_hh.T[:, _GATE_PERM].astype(BF16)
    wc[H:H + E] = W_ih.T[:, _GATE_PERM].astype(BF16)
    wc[H + E] = (b_ih + b_hh)[_GATE_PERM].astype(BF16)
    emb_bf = emb.astype(BF16)

    # --- word assignment ---------------------------------------------------
    plan = _plan(lengths_np)
    blocks, entries = plan["blocks"], plan["entries"]

    sig = (tuple((b["L"], b["dump_c"]) for b in blocks), tuple(entries))
    key = hash(sig)
    if key not in _PROGRAM_CACHE:
        _PROGRAM_CACHE[key] = _build_program(sig, blocks, entries)
    nc = _PROGRAM_CACHE[key]

    xss = _build_xslabs(plan, chars, lengths_np, emb_bf)
    in_maps = [{"xs": xss[c], "wc": wc} for c in range(NCORES)]

    res = run_bass_kernel_spmd(nc, in_maps, core_ids=list(range(NCORES)))
    kernel._last_nc = nc
    kernel._last_in_maps = in_maps

    # --- gather results ----------------------------------------------------
    outs = np.stack([np.asarray(r["out"]) for r in res.results])
    cvs = np.stack([np.asarray(r["cv"]) for r in res.results])

    result = np.empty((n, H), dtype=np.float32)
    for c in range(NCORES):
        for bi in range(len(blocks)):
            words = plan["assign"][c][bi]
            valid = words >= 0
            if not valid.any():
                continue
            w = words[valid]
            cols = np.nonzero(valid)[0]
            result[w] = outs[c, bi, :, cols].astype(np.float32)
    _host_finish(plan["foreign"], outs, cvs, emb, W_ih, W_hh, b_ih, b_hh,
                 chars, lengths_np, result)
    return result
